# revision 15
# baseline (speedup 1.0000x reference)
"""Trainium2 Bass kernel for nn_Attn_Head (GNN attention head).

Computation (reference):
    seq_fts = x @ W1.T                      # [N, 64]
    f1 = seq_fts @ a1 ; f2 = seq_fts @ a2   # [N]
    logits[i, j] = leaky_relu(f1[j] + f2[i], 0.01)
    coefs = softmax(logits + bias_mx, axis=0)   # per-column softmax over i
    out = elu(coefs @ seq_fts)[None]        # [1, N, 64]

Sharding: columns j of the softmax matrix are block-sharded across the 8
NeuronCores (1024 columns each). The device works on the TRANSPOSED
matrix (tiles [j_partitions, i_free]) so that:
  - the softmax reduction over i runs along the free axis (fused into the
    Exp activation via accum_out),
  - the output matmul retT[c,i] = sum_j sf[j,c]*coefs[i,j] contracts over
    j on the partition axis with the small seq_fts block as the
    stationary operand (few LDWEIGHTS, 512-wide streaming).

The host-side transposition pass of the bias matrix (which the baseline
already used to fold the rank-1 term 0.01*(f1[j]+f2[i])) now folds the
whole leaky_relu of the rank-1 logit matrix:
    expoT[j, i] = bias[i, j] + 0.01*z + 0.99*relu(z),  z = f1[j]+f2[i]
so each core streams its exponent row-block ONCE and runs the softmax
(Exp with fp32 row-sum accumulation, normalizers folded into the
stationary seq_fts weights) plus both matmuls on device.

Precision (hw rel err ~1.2e-3, gate 2e-2): the exponent matrix is
shipped fp16 (halves the dominant HBM stream); Exp emits bf16 (values
up to ~2e5 exceed fp16 range), and both matmul operands are bf16 (full
PE rate vs fp32's two-pass half-rate mode). PSUM accumulation stays
fp32 and the partial retT [64, 8192] leaves each core fp32; the host
sums the 8 partials, transposes, and applies the final elu.
"""

import sys

for _p in ("/opt/trn_rl_repo", "/root/.axon_site/_ro/trn_rl_repo"):
    if _p not in sys.path:
        sys.path.insert(0, _p)

import numpy as np
import ml_dtypes

import concourse.bass as bass
import concourse.tile as tile
from concourse import mybir
from concourse.bass_utils import run_bass_kernel_spmd

N = 8192          # nodes
C = 256           # input channels
D = 64            # output size
NCORES = 8
B = N // NCORES   # columns per core (1024)
P = 128           # partitions
Q = B // P        # j-chunks per core (8)
HALF = N // 2     # i-subtile width (4096)
QTR = N // 4      # row-0 exp granularity
SEG = 512         # matmul streaming width
F32 = mybir.dt.float32
F16 = mybir.dt.float16
BF16 = mybir.dt.bfloat16
NP_BF16 = ml_dtypes.bfloat16


# ---------------------------------------------------------------------------
# Workaround: this walrus build rejects more than ONE sem-wait per
# instruction ("Too many sync wait commands"). After Tile lowering, split
# any instruction carrying k>1 waits into (k-1) single-wait NOPs on the
# same engine placed immediately before it — semantically identical, since
# an engine's sequencer processes waits in stream order.
def _split_multiwaits(nc):
    n_split = 0
    for f in nc.m.functions:
        for bb in f.blocks:
            insts = bb.instructions
            out = []
            for inst in insts:
                si = inst.sync_info
                if si is not None and si.on_wait and len(si.on_wait) > 1:
                    waits = list(si.on_wait)
                    for k, w in enumerate(waits[:-1]):
                        nop = mybir.InstNoOp(
                            name=f"{inst.name}.wsplit{k}", ins=[], outs=[]
                        )
                        nop.engine = inst.engine
                        nop.sync_info = mybir.SyncInfo(on_wait=[w], on_update=[])
                        out.append(nop)
                        n_split += 1
                    inst.sync_info = mybir.SyncInfo(
                        on_wait=[waits[-1]], on_update=list(si.on_update)
                    )
                out.append(inst)
            if len(out) != len(insts):
                bb.instructions = out
    return n_split
# ---------------------------------------------------------------------------


def build_nc(bias_bufs: int = 4, e_bufs: int = 2,
             split_multiwaits: bool = True):
    """Build the per-core Bass program (SPMD: same program on all cores)."""
    nc = bass.Bass("TRN2", target_bir_lowering=False, debug=False,
                   num_devices=NCORES)

    # Per-core inputs.
    expoP = nc.dram_tensor("expoP", [B, N], F16, kind="ExternalInput")
    xT = nc.dram_tensor("xT", [C, B], BF16, kind="ExternalInput")
    w1T = nc.dram_tensor("w1T", [C, D], BF16, kind="ExternalInput")
    # fp16 partials: each core's retT entries are softmax-weighted averages
    # (small magnitude), so fp16 costs ~5e-5 extra rel err and halves the
    # output stream.
    ret = nc.dram_tensor("ret", [D, N], F16, kind="ExternalOutput")  # retT

    with tile.TileContext(nc) as tc:
        with (
            tc.tile_pool(name="singles", bufs=1) as singles,
            tc.tile_pool(name="bias", bufs=bias_bufs) as bias_pool,
            tc.tile_pool(name="e", bufs=e_bufs) as e_pool,
            tc.tile_pool(name="psum", bufs=1, space="PSUM") as psum_pool,
        ):
            # --- setup loads. Rows 0/1 of the exponent matrix go first (in
            # chunks, so the first Exps start as early as possible); only
            # then xT/w1T — the seq_fts matmuls don't bind until ~20us.
            b0_sb = bias_pool.tile([P, N], F16, tag="bias")
            for c0, c1 in ((0, 1024), (1024, 2048), (2048, 4096), (4096, N)):
                nc.sync.dma_start(
                    out=b0_sb[:, c0:c1], in_=expoP[0:P, c0:c1]
                )
            b1_sb = bias_pool.tile([P, N], F16, tag="bias")
            for h in range(2):
                nc.sync.dma_start(
                    out=b1_sb[:, h * HALF:(h + 1) * HALF],
                    in_=expoP[P:2 * P, h * HALF:(h + 1) * HALF],
                )
            w1T_sb = singles.tile([P, 2, D], BF16)     # k-chunks of W1.T
            nc.sync.dma_start(
                out=w1T_sb, in_=w1T[:, :].rearrange("(k p) d -> p k d", p=P)
            )
            xT_sb = singles.tile([P, 2, B], BF16)      # k-chunks of x_blk.T
            nc.sync.dma_start(
                out=xT_sb, in_=xT[:, :].rearrange("(k p) b -> p k b", p=P)
            )

            # --- PSUM: retT [64, 8192] as 16 [64, 512] regions:
            # seg s<8  -> partitions 0:64,   bank s
            # seg s>=8 -> partitions 64:128, bank s-8
            ret_ps = psum_pool.tile([P, 8 * SEG], F32)

            def seg_out(s):
                if s < 8:
                    return ret_ps[0:D, s * SEG:(s + 1) * SEG], None
                return ret_ps[D:P, (s - 8) * SEG:(s - 7) * SEG], (0, 64)

            # --- seq_fts block: sf[jl, c] for this core's 1024 columns.
            # Borrows ret_ps[:, 0:64] before the main accumulation starts.
            # Evacuation copies on the DVE (idle this early); ACT must stay
            # free for the Exp stream.
            sf_all = singles.tile([P, Q * D], F32)
            for qq in range(Q):
                for kc in range(2):
                    nc.tensor.matmul(
                        ret_ps[:, 0:D],
                        lhsT=xT_sb[:, kc, qq * P:(qq + 1) * P],
                        rhs=w1T_sb[:, kc, :],
                        start=(kc == 0),
                        stop=(kc == 1),
                    )
                nc.vector.tensor_copy(sf_all[:, qq * D:(qq + 1) * D],
                                      ret_ps[:, 0:D])

            sf_scaled = singles.tile([P, Q * D], BF16)
            s8 = singles.tile([P, 8], F32)         # split-row partial sums
            s2 = singles.tile([P, 2], F32)
            sq = singles.tile([P, Q], F32)         # row sums
            rinv = singles.tile([P, Q], F32)       # reciprocals
            # retT_sb[p,:]: p<64 -> retT[p, 0:4096]; p>=64 -> retT[p-64, 4096:]
            ret_sb = singles.tile([P, 8 * SEG], F16)

            # Early rows run Exp at sub-row granularity (start on the first
            # DMA'd chunk); once the DMA prefetch is ahead of the Exp train,
            # one full-row Exp per row has the least overhead.
            # chunks[q]: (col ranges, s8 accum slot base) or None for full.
            chunks = {
                0: ([(0, 1024), (1024, 2048), (2048, 4096), (4096, N)], 0),
                1: ([(0, HALF), (HALF, N)], 4),
                2: ([(0, HALF), (HALF, N)], 6),
            }

            def exp_row(q, b_sb, e_sb):
                if q not in chunks:
                    nc.scalar.activation(
                        out=e_sb, in_=b_sb,
                        func=mybir.ActivationFunctionType.Exp,
                        accum_out=sq[:, q:q + 1],
                    )
                    return
                ranges, base = chunks[q]
                for c, (c0, c1) in enumerate(ranges):
                    nc.scalar.activation(
                        out=e_sb[:, c0:c1], in_=b_sb[:, c0:c1],
                        func=mybir.ActivationFunctionType.Exp,
                        accum_out=s8[:, base + c:base + c + 1],
                    )
                if len(ranges) == 4:
                    nc.vector.tensor_add(s2[:, 0:1], s8[:, base:base + 1],
                                         s8[:, base + 1:base + 2])
                    nc.vector.tensor_add(s2[:, 1:2], s8[:, base + 2:base + 3],
                                         s8[:, base + 3:base + 4])
                    nc.vector.tensor_add(sq[:, q:q + 1], s2[:, 0:1],
                                         s2[:, 1:2])
                else:
                    nc.vector.tensor_add(sq[:, q:q + 1], s8[:, base:base + 1],
                                         s8[:, base + 1:base + 2])

            # --- main loop over j-chunks --------------------------------
            for q in range(Q):
                if q == 0:
                    b_sb = b0_sb
                elif q == 1:
                    b_sb = b1_sb
                else:
                    b_sb = bias_pool.tile([P, N], F16, tag="bias")
                    for h in range(2):
                        nc.sync.dma_start(
                            out=b_sb[:, h * HALF:(h + 1) * HALF],
                            in_=expoP[q * P:(q + 1) * P,
                                      h * HALF:(h + 1) * HALF],
                        )

                e_sb = e_pool.tile([P, N], BF16, tag="e")
                exp_row(q, b_sb, e_sb)

                # normalizer -> fold into the stationary seq_fts weights
                nc.vector.reciprocal(rinv[:, q:q + 1], sq[:, q:q + 1])
                nc.vector.tensor_scalar_mul(
                    sf_scaled[:, q * D:(q + 1) * D],
                    sf_all[:, q * D:(q + 1) * D],
                    rinv[:, q:q + 1],
                )

                # retT[seg] += sf_scaled[q].T @ e[seg]   (sf stationary).
                # On the last q, process segments low/high interleaved,
                # evacuate each PSUM segment right after its final matmul,
                # and kick output DMAs (covering BOTH partition ranges, so
                # both DMA-engine groups stream) halfway and at the end.
                seg_order = (list(range(16)) if q != Q - 1 else
                             [0, 8, 1, 9, 2, 10, 3, 11,
                              4, 12, 5, 13, 6, 14, 7, 15])
                for k, s in enumerate(seg_order):
                    out_ap, tpos = seg_out(s)
                    nc.tensor.matmul(
                        out_ap,
                        lhsT=sf_scaled[:, q * D:(q + 1) * D],
                        rhs=e_sb[:, s * SEG:(s + 1) * SEG],
                        start=(q == 0),
                        stop=(q == Q - 1),
                        tile_position=tpos,
                    )
                    if q == Q - 1:
                        dst = (ret_sb[0:D, s * SEG:(s + 1) * SEG] if s < 8
                               else ret_sb[D:P, (s - 8) * SEG:(s - 7) * SEG])
                        if k % 2 == 0:
                            nc.scalar.copy(out=dst, in_=out_ap)
                        else:
                            nc.vector.tensor_copy(dst, out_ap)
                        if k in (7, 15):
                            c0 = 0 if k == 7 else 4 * SEG
                            nc.sync.dma_start(
                                out=ret[:, c0:c0 + 4 * SEG],
                                in_=ret_sb[0:D, c0:c0 + 4 * SEG],
                            )
                            nc.sync.dma_start(
                                out=ret[:, 8 * SEG + c0:8 * SEG + c0 + 4 * SEG],
                                in_=ret_sb[D:P, c0:c0 + 4 * SEG],
                            )

    if split_multiwaits:
        _split_multiwaits(nc)
    return nc


_NC_CACHE = None


def _get_nc():
    global _NC_CACHE
    if _NC_CACHE is None:
        _NC_CACHE = build_nc()
    return _NC_CACHE


def host_prep(x, bias_mx, W1, a1, a2):
    """Shard + lay out inputs for the 8 cores.

    The transposition pass over each core's bias block folds the whole
    elementwise leaky_relu(f1[j]+f2[i]) logit term, producing the softmax
    exponent matrix the device streams.
    """
    x = np.ascontiguousarray(x, dtype=np.float32)
    W1 = np.ascontiguousarray(W1, dtype=np.float32)
    sf_host = x @ W1.T                   # only used for f1/f2 (logit fold)
    f1 = sf_host @ np.asarray(a1, dtype=np.float32)
    f2 = sf_host @ np.asarray(a2, dtype=np.float32)

    w1T = np.ascontiguousarray(W1.T.astype(NP_BF16))
    in_maps = []
    for d in range(NCORES):
        j0 = d * B
        blk = bias_mx[:, j0:j0 + B]
        z = f1[j0:j0 + B][:, None] + f2[None, :]
        expoP = np.empty((B, N), dtype=np.float32)
        np.copyto(expoP, blk.T)
        expoP += 0.01 * z
        expoP += 0.99 * np.maximum(z, 0.0)
        in_maps.append({
            "expoP": expoP.astype(np.float16),
            "xT": np.ascontiguousarray(x[j0:j0 + B].T.astype(NP_BF16)),
            "w1T": w1T,
        })
    return in_maps


def postprocess(results):
    retT = results[0]["ret"].astype(np.float32)
    for d in range(1, NCORES):
        retT = retT + results[d]["ret"].astype(np.float32)
    r = retT.T
    out = np.where(r > 0.0, r, np.expm1(np.minimum(r, 0.0)))
    return np.ascontiguousarray(out[None], dtype=np.float32)


def kernel(x, bias_mx, W1, a1, a2):
    nc = _get_nc()
    in_maps = host_prep(x, bias_mx, W1, a1, a2)
    res = run_bass_kernel_spmd(nc, in_maps, list(range(NCORES)))
    return postprocess(res.results)


if __name__ == "__main__":
    rng = np.random.default_rng(0)
    x = rng.standard_normal((N, C), dtype=np.float32)
    bias_mx = rng.standard_normal((N, N), dtype=np.float32)
    W1 = rng.standard_normal((D, C), dtype=np.float32) / np.sqrt(C)
    a1 = rng.standard_normal(D).astype(np.float32) / np.sqrt(D)
    a2 = rng.standard_normal(D).astype(np.float32) / np.sqrt(D)
    out = kernel(x=x, bias_mx=bias_mx, W1=W1, a1=a1, a2=a2)
    print("out", out.shape, out.dtype, float(np.abs(out).max()))


# revision 16
# speedup vs baseline: 1.0415x; 1.0415x over previous
"""Trainium2 Bass kernel for nn_Attn_Head (GNN attention head).

Computation (reference):
    seq_fts = x @ W1.T                      # [N, 64]
    f1 = seq_fts @ a1 ; f2 = seq_fts @ a2   # [N]
    logits[i, j] = leaky_relu(f1[j] + f2[i], 0.01)
    coefs = softmax(logits + bias_mx, axis=0)   # per-column softmax over i
    out = elu(coefs @ seq_fts)[None]        # [1, N, 64]

Sharding: columns j of the softmax matrix are block-sharded across the 8
NeuronCores (1024 columns each). The device works on the TRANSPOSED
matrix (tiles [j_partitions, i_free]) so that:
  - the softmax reduction over i runs along the free axis (fused into the
    Exp activation via accum_out),
  - the output matmul retT[c,i] = sum_j sf[j,c]*coefs[i,j] contracts over
    j on the partition axis with the small seq_fts block as the
    stationary operand (few LDWEIGHTS, 512-wide streaming).

The host-side transposition pass of the bias matrix (which the baseline
already used to fold the rank-1 term 0.01*(f1[j]+f2[i])) now folds the
whole leaky_relu of the rank-1 logit matrix:
    expoT[j, i] = bias[i, j] + 0.01*z + 0.99*relu(z),  z = f1[j]+f2[i]
so each core streams its exponent row-block ONCE and runs the softmax
(Exp with fp32 row-sum accumulation, normalizers folded into the
stationary seq_fts weights) plus both matmuls on device.

Precision (hw rel err ~1.2e-3, gate 2e-2): the exponent matrix is
shipped fp16 (halves the dominant HBM stream); Exp emits bf16 (values
up to ~2e5 exceed fp16 range), and both matmul operands are bf16 (full
PE rate vs fp32's two-pass half-rate mode). PSUM accumulation stays
fp32 and the partial retT [64, 8192] leaves each core fp32; the host
sums the 8 partials, transposes, and applies the final elu.
"""

import sys

for _p in ("/opt/trn_rl_repo", "/root/.axon_site/_ro/trn_rl_repo"):
    if _p not in sys.path:
        sys.path.insert(0, _p)

import numpy as np
import ml_dtypes

import concourse.bass as bass
import concourse.tile as tile
from concourse import mybir
from concourse.bass_utils import run_bass_kernel_spmd

N = 8192          # nodes
C = 256           # input channels
D = 64            # output size
NCORES = 8
B = N // NCORES   # columns per core (1024)
P = 128           # partitions
Q = B // P        # j-chunks per core (8)
HALF = N // 2     # i-subtile width (4096)
QTR = N // 4      # row-0 exp granularity
SEG = 512         # matmul streaming width
F32 = mybir.dt.float32
F16 = mybir.dt.float16
BF16 = mybir.dt.bfloat16
NP_BF16 = ml_dtypes.bfloat16


# ---------------------------------------------------------------------------
# Workaround: this walrus build rejects more than ONE sem-wait per
# instruction ("Too many sync wait commands"). After Tile lowering, split
# any instruction carrying k>1 waits into (k-1) single-wait NOPs on the
# same engine placed immediately before it — semantically identical, since
# an engine's sequencer processes waits in stream order.
def _split_multiwaits(nc):
    n_split = 0
    for f in nc.m.functions:
        for bb in f.blocks:
            insts = bb.instructions
            out = []
            for inst in insts:
                si = inst.sync_info
                if si is not None and si.on_wait and len(si.on_wait) > 1:
                    waits = list(si.on_wait)
                    for k, w in enumerate(waits[:-1]):
                        nop = mybir.InstNoOp(
                            name=f"{inst.name}.wsplit{k}", ins=[], outs=[]
                        )
                        nop.engine = inst.engine
                        nop.sync_info = mybir.SyncInfo(on_wait=[w], on_update=[])
                        out.append(nop)
                        n_split += 1
                    inst.sync_info = mybir.SyncInfo(
                        on_wait=[waits[-1]], on_update=list(si.on_update)
                    )
                out.append(inst)
            if len(out) != len(insts):
                bb.instructions = out
    return n_split
# ---------------------------------------------------------------------------


def build_nc(bias_bufs: int = 4, e_bufs: int = 3,
             split_multiwaits: bool = True):
    """Build the per-core Bass program (SPMD: same program on all cores)."""
    nc = bass.Bass("TRN2", target_bir_lowering=False, debug=False,
                   num_devices=NCORES)

    # Per-core inputs.
    expoP = nc.dram_tensor("expoP", [B, N], F16, kind="ExternalInput")
    xT = nc.dram_tensor("xT", [C, B], BF16, kind="ExternalInput")
    w1T = nc.dram_tensor("w1T", [C, D], BF16, kind="ExternalInput")
    # fp16 partials: each core's retT entries are softmax-weighted averages
    # (small magnitude), so fp16 costs ~5e-5 extra rel err and halves the
    # output stream.
    ret = nc.dram_tensor("ret", [D, N], F16, kind="ExternalOutput")  # retT

    with tile.TileContext(nc) as tc:
        with (
            tc.tile_pool(name="singles", bufs=1) as singles,
            tc.tile_pool(name="bias", bufs=bias_bufs) as bias_pool,
            tc.tile_pool(name="e", bufs=e_bufs) as e_pool,
            tc.tile_pool(name="psum", bufs=1, space="PSUM") as psum_pool,
        ):
            # --- setup loads. Rows 0/1 of the exponent matrix go first (in
            # chunks, so the first Exps start as early as possible); only
            # then xT/w1T — the seq_fts matmuls don't bind until ~20us.
            b0_sb = bias_pool.tile([P, N], F16, tag="bias")
            for c0, c1 in ((0, 1024), (1024, 2048), (2048, 4096), (4096, N)):
                nc.sync.dma_start(
                    out=b0_sb[:, c0:c1], in_=expoP[0:P, c0:c1]
                )
            b1_sb = bias_pool.tile([P, N], F16, tag="bias")
            for h in range(2):
                nc.sync.dma_start(
                    out=b1_sb[:, h * HALF:(h + 1) * HALF],
                    in_=expoP[P:2 * P, h * HALF:(h + 1) * HALF],
                )
            w1T_sb = singles.tile([P, 2, D], BF16)     # k-chunks of W1.T
            nc.sync.dma_start(
                out=w1T_sb, in_=w1T[:, :].rearrange("(k p) d -> p k d", p=P)
            )
            xT_sb = singles.tile([P, 2, B], BF16)      # k-chunks of x_blk.T
            nc.sync.dma_start(
                out=xT_sb, in_=xT[:, :].rearrange("(k p) b -> p k b", p=P)
            )

            # --- PSUM: retT [64, 8192] as 16 [64, 512] regions:
            # seg s<8  -> partitions 0:64,   bank s
            # seg s>=8 -> partitions 64:128, bank s-8
            ret_ps = psum_pool.tile([P, 8 * SEG], F32)

            def seg_out(s):
                if s < 8:
                    return ret_ps[0:D, s * SEG:(s + 1) * SEG], None
                return ret_ps[D:P, (s - 8) * SEG:(s - 7) * SEG], (0, 64)

            # --- seq_fts block: sf[jl, c] for this core's 1024 columns.
            # Borrows ret_ps[:, 0:64] before the main accumulation starts.
            # Evacuation copies on the DVE (idle this early); ACT must stay
            # free for the Exp stream.
            sf_all = singles.tile([P, Q * D], F32)
            for qq in range(Q):
                for kc in range(2):
                    nc.tensor.matmul(
                        ret_ps[:, 0:D],
                        lhsT=xT_sb[:, kc, qq * P:(qq + 1) * P],
                        rhs=w1T_sb[:, kc, :],
                        start=(kc == 0),
                        stop=(kc == 1),
                    )
                nc.vector.tensor_copy(sf_all[:, qq * D:(qq + 1) * D],
                                      ret_ps[:, 0:D])

            sf_scaled = singles.tile([P, Q * D], BF16)
            s8 = singles.tile([P, 8], F32)         # split-row partial sums
            s2 = singles.tile([P, 2], F32)
            sq = singles.tile([P, Q], F32)         # row sums
            rinv = singles.tile([P, Q], F32)       # reciprocals
            # retT_sb[p,:]: p<64 -> retT[p, 0:4096]; p>=64 -> retT[p-64, 4096:]
            ret_sb = singles.tile([P, 8 * SEG], F16)

            # Early rows run Exp at sub-row granularity (start on the first
            # DMA'd chunk); once the DMA prefetch is ahead of the Exp train,
            # one full-row Exp per row has the least overhead.
            # chunks[q]: (col ranges, s8 accum slot base) or None for full.
            chunks = {
                0: ([(0, 1024), (1024, 2048), (2048, 4096), (4096, N)], 0),
                1: ([(0, HALF), (HALF, N)], 4),
                2: ([(0, HALF), (HALF, N)], 6),
            }

            def exp_row(q, b_sb, e_sb):
                if q not in chunks:
                    nc.scalar.activation(
                        out=e_sb, in_=b_sb,
                        func=mybir.ActivationFunctionType.Exp,
                        accum_out=sq[:, q:q + 1],
                    )
                    return
                ranges, base = chunks[q]
                for c, (c0, c1) in enumerate(ranges):
                    nc.scalar.activation(
                        out=e_sb[:, c0:c1], in_=b_sb[:, c0:c1],
                        func=mybir.ActivationFunctionType.Exp,
                        accum_out=s8[:, base + c:base + c + 1],
                    )
                if len(ranges) == 4:
                    nc.vector.tensor_add(s2[:, 0:1], s8[:, base:base + 1],
                                         s8[:, base + 1:base + 2])
                    nc.vector.tensor_add(s2[:, 1:2], s8[:, base + 2:base + 3],
                                         s8[:, base + 3:base + 4])
                    nc.vector.tensor_add(sq[:, q:q + 1], s2[:, 0:1],
                                         s2[:, 1:2])
                else:
                    nc.vector.tensor_add(sq[:, q:q + 1], s8[:, base:base + 1],
                                         s8[:, base + 1:base + 2])

            # --- main loop over j-chunks --------------------------------
            for q in range(Q):
                if q == 0:
                    b_sb = b0_sb
                elif q == 1:
                    b_sb = b1_sb
                else:
                    b_sb = bias_pool.tile([P, N], F16, tag="bias")
                    for h in range(2):
                        nc.sync.dma_start(
                            out=b_sb[:, h * HALF:(h + 1) * HALF],
                            in_=expoP[q * P:(q + 1) * P,
                                      h * HALF:(h + 1) * HALF],
                        )

                e_sb = e_pool.tile([P, N], BF16, tag="e")
                exp_row(q, b_sb, e_sb)

                # normalizer -> fold into the stationary seq_fts weights
                nc.vector.reciprocal(rinv[:, q:q + 1], sq[:, q:q + 1])
                nc.vector.tensor_scalar_mul(
                    sf_scaled[:, q * D:(q + 1) * D],
                    sf_all[:, q * D:(q + 1) * D],
                    rinv[:, q:q + 1],
                )

                # retT[seg] += sf_scaled[q].T @ e[seg]   (sf stationary).
                # On the last q, process segments low/high interleaved,
                # evacuate each PSUM segment right after its final matmul,
                # and kick output DMAs (covering BOTH partition ranges, so
                # both DMA-engine groups stream) halfway and at the end.
                seg_order = (list(range(16)) if q != Q - 1 else
                             [0, 8, 1, 9, 2, 10, 3, 11,
                              4, 12, 5, 13, 6, 14, 7, 15])
                for k, s in enumerate(seg_order):
                    out_ap, tpos = seg_out(s)
                    nc.tensor.matmul(
                        out_ap,
                        lhsT=sf_scaled[:, q * D:(q + 1) * D],
                        rhs=e_sb[:, s * SEG:(s + 1) * SEG],
                        start=(q == 0),
                        stop=(q == Q - 1),
                        tile_position=tpos,
                    )
                    if q == Q - 1:
                        dst = (ret_sb[0:D, s * SEG:(s + 1) * SEG] if s < 8
                               else ret_sb[D:P, (s - 8) * SEG:(s - 7) * SEG])
                        if k % 2 == 0:
                            nc.scalar.copy(out=dst, in_=out_ap)
                        else:
                            nc.vector.tensor_copy(dst, out_ap)
                        if k in (7, 15):
                            c0 = 0 if k == 7 else 4 * SEG
                            nc.sync.dma_start(
                                out=ret[:, c0:c0 + 4 * SEG],
                                in_=ret_sb[0:D, c0:c0 + 4 * SEG],
                            )
                            nc.sync.dma_start(
                                out=ret[:, 8 * SEG + c0:8 * SEG + c0 + 4 * SEG],
                                in_=ret_sb[D:P, c0:c0 + 4 * SEG],
                            )

    if split_multiwaits:
        _split_multiwaits(nc)
    return nc


_NC_CACHE = None


def _get_nc():
    global _NC_CACHE
    if _NC_CACHE is None:
        _NC_CACHE = build_nc()
    return _NC_CACHE


def host_prep(x, bias_mx, W1, a1, a2):
    """Shard + lay out inputs for the 8 cores.

    The transposition pass over each core's bias block folds the whole
    elementwise leaky_relu(f1[j]+f2[i]) logit term, producing the softmax
    exponent matrix the device streams.
    """
    x = np.ascontiguousarray(x, dtype=np.float32)
    W1 = np.ascontiguousarray(W1, dtype=np.float32)
    sf_host = x @ W1.T                   # only used for f1/f2 (logit fold)
    f1 = sf_host @ np.asarray(a1, dtype=np.float32)
    f2 = sf_host @ np.asarray(a2, dtype=np.float32)

    w1T = np.ascontiguousarray(W1.T.astype(NP_BF16))
    in_maps = []
    for d in range(NCORES):
        j0 = d * B
        blk = bias_mx[:, j0:j0 + B]
        z = f1[j0:j0 + B][:, None] + f2[None, :]
        expoP = np.empty((B, N), dtype=np.float32)
        np.copyto(expoP, blk.T)
        expoP += 0.01 * z
        expoP += 0.99 * np.maximum(z, 0.0)
        in_maps.append({
            "expoP": expoP.astype(np.float16),
            "xT": np.ascontiguousarray(x[j0:j0 + B].T.astype(NP_BF16)),
            "w1T": w1T,
        })
    return in_maps


def postprocess(results):
    retT = results[0]["ret"].astype(np.float32)
    for d in range(1, NCORES):
        retT = retT + results[d]["ret"].astype(np.float32)
    r = retT.T
    out = np.where(r > 0.0, r, np.expm1(np.minimum(r, 0.0)))
    return np.ascontiguousarray(out[None], dtype=np.float32)


def kernel(x, bias_mx, W1, a1, a2):
    nc = _get_nc()
    in_maps = host_prep(x, bias_mx, W1, a1, a2)
    res = run_bass_kernel_spmd(nc, in_maps, list(range(NCORES)))
    return postprocess(res.results)


if __name__ == "__main__":
    rng = np.random.default_rng(0)
    x = rng.standard_normal((N, C), dtype=np.float32)
    bias_mx = rng.standard_normal((N, N), dtype=np.float32)
    W1 = rng.standard_normal((D, C), dtype=np.float32) / np.sqrt(C)
    a1 = rng.standard_normal(D).astype(np.float32) / np.sqrt(D)
    a2 = rng.standard_normal(D).astype(np.float32) / np.sqrt(D)
    out = kernel(x=x, bias_mx=bias_mx, W1=W1, a1=a1, a2=a2)
    print("out", out.shape, out.dtype, float(np.abs(out).max()))


# revision 19
# speedup vs baseline: 1.0462x; 1.0045x over previous
"""Trainium2 Bass kernel for nn_Attn_Head (GNN attention head).

Computation (reference):
    seq_fts = x @ W1.T                      # [N, 64]
    f1 = seq_fts @ a1 ; f2 = seq_fts @ a2   # [N]
    logits[i, j] = leaky_relu(f1[j] + f2[i], 0.01)
    coefs = softmax(logits + bias_mx, axis=0)   # per-column softmax over i
    out = elu(coefs @ seq_fts)[None]        # [1, N, 64]

Sharding: columns j of the softmax matrix are block-sharded across the 8
NeuronCores (1024 columns each). The device works on the TRANSPOSED
matrix (tiles [j_partitions, i_free]) so that:
  - the softmax reduction over i runs along the free axis (fused into the
    Exp activation via accum_out),
  - the output matmul retT[c,i] = sum_j sf[j,c]*coefs[i,j] contracts over
    j on the partition axis with the small seq_fts block as the
    stationary operand (few LDWEIGHTS, 512-wide streaming).

The host-side transposition pass of the bias matrix (which the baseline
already used to fold the rank-1 term 0.01*(f1[j]+f2[i])) now folds the
whole leaky_relu of the rank-1 logit matrix:
    expoT[j, i] = bias[i, j] + 0.01*z + 0.99*relu(z),  z = f1[j]+f2[i]
so each core streams its exponent row-block ONCE and runs the softmax
(Exp with fp32 row-sum accumulation, normalizers folded into the
stationary seq_fts weights) plus both matmuls on device.

Precision (hw rel err ~1.2e-3, gate 2e-2): the exponent matrix is
shipped fp16 (halves the dominant HBM stream); Exp emits bf16 (values
up to ~2e5 exceed fp16 range), and both matmul operands are bf16 (full
PE rate vs fp32's two-pass half-rate mode). PSUM accumulation stays
fp32 and the partial retT [64, 8192] leaves each core fp32; the host
sums the 8 partials, transposes, and applies the final elu.
"""

import sys

for _p in ("/opt/trn_rl_repo", "/root/.axon_site/_ro/trn_rl_repo"):
    if _p not in sys.path:
        sys.path.insert(0, _p)

import numpy as np
import ml_dtypes

import concourse.bass as bass
import concourse.tile as tile
from concourse import mybir
from concourse.bass_utils import run_bass_kernel_spmd

N = 8192          # nodes
C = 256           # input channels
D = 64            # output size
NCORES = 8
B = N // NCORES   # columns per core (1024)
P = 128           # partitions
Q = B // P        # j-chunks per core (8)
HALF = N // 2     # i-subtile width (4096)
QTR = N // 4      # row-0 exp granularity
SEG = 512         # matmul streaming width
F32 = mybir.dt.float32
F16 = mybir.dt.float16
BF16 = mybir.dt.bfloat16
NP_BF16 = ml_dtypes.bfloat16


# ---------------------------------------------------------------------------
# Workaround: this walrus build rejects more than ONE sem-wait per
# instruction ("Too many sync wait commands"). After Tile lowering, split
# any instruction carrying k>1 waits into (k-1) single-wait NOPs on the
# same engine placed immediately before it — semantically identical, since
# an engine's sequencer processes waits in stream order.
def _split_multiwaits(nc):
    n_split = 0
    for f in nc.m.functions:
        for bb in f.blocks:
            insts = bb.instructions
            out = []
            for inst in insts:
                si = inst.sync_info
                if si is not None and si.on_wait and len(si.on_wait) > 1:
                    waits = list(si.on_wait)
                    for k, w in enumerate(waits[:-1]):
                        nop = mybir.InstNoOp(
                            name=f"{inst.name}.wsplit{k}", ins=[], outs=[]
                        )
                        nop.engine = inst.engine
                        nop.sync_info = mybir.SyncInfo(on_wait=[w], on_update=[])
                        out.append(nop)
                        n_split += 1
                    inst.sync_info = mybir.SyncInfo(
                        on_wait=[waits[-1]], on_update=list(si.on_update)
                    )
                out.append(inst)
            if len(out) != len(insts):
                bb.instructions = out
    return n_split
# ---------------------------------------------------------------------------


def build_nc(bias_bufs: int = 4, e_bufs: int = 3,
             split_multiwaits: bool = True):
    """Build the per-core Bass program (SPMD: same program on all cores)."""
    nc = bass.Bass("TRN2", target_bir_lowering=False, debug=False,
                   num_devices=NCORES)

    # Per-core inputs.
    expoP = nc.dram_tensor("expoP", [B, N], F16, kind="ExternalInput")
    xT = nc.dram_tensor("xT", [C, B], BF16, kind="ExternalInput")
    w1T = nc.dram_tensor("w1T", [C, D], BF16, kind="ExternalInput")
    # fp16 partials: each core's retT entries are softmax-weighted averages
    # (small magnitude), so fp16 costs ~5e-5 extra rel err and halves the
    # output stream.
    ret = nc.dram_tensor("ret", [D, N], F16, kind="ExternalOutput")  # retT

    with tile.TileContext(nc) as tc:
        with (
            tc.tile_pool(name="singles", bufs=1) as singles,
            tc.tile_pool(name="bias", bufs=bias_bufs) as bias_pool,
            tc.tile_pool(name="e", bufs=e_bufs) as e_pool,
            tc.tile_pool(name="psum", bufs=1, space="PSUM") as psum_pool,
        ):
            # --- setup loads. Rows 0/1 of the exponent matrix go first (in
            # chunks, so the first Exps start as early as possible); only
            # then xT/w1T — the seq_fts matmuls don't bind until ~20us.
            b0_sb = bias_pool.tile([P, N], F16, tag="bias")
            for c in range(8):
                nc.sync.dma_start(
                    out=b0_sb[:, c * 1024:(c + 1) * 1024],
                    in_=expoP[0:P, c * 1024:(c + 1) * 1024],
                )
            b1_sb = bias_pool.tile([P, N], F16, tag="bias")
            for c in range(4):
                nc.sync.dma_start(
                    out=b1_sb[:, c * QTR:(c + 1) * QTR],
                    in_=expoP[P:2 * P, c * QTR:(c + 1) * QTR],
                )
            w1T_sb = singles.tile([P, 2, D], BF16)     # k-chunks of W1.T
            nc.sync.dma_start(
                out=w1T_sb, in_=w1T[:, :].rearrange("(k p) d -> p k d", p=P)
            )
            xT_sb = singles.tile([P, 2, B], BF16)      # k-chunks of x_blk.T
            nc.sync.dma_start(
                out=xT_sb, in_=xT[:, :].rearrange("(k p) b -> p k b", p=P)
            )

            # --- PSUM: retT [64, 8192] as 16 [64, 512] regions:
            # seg s<8  -> partitions 0:64,   bank s
            # seg s>=8 -> partitions 64:128, bank s-8
            ret_ps = psum_pool.tile([P, 8 * SEG], F32)

            def seg_out(s):
                if s < 8:
                    return ret_ps[0:D, s * SEG:(s + 1) * SEG], None
                return ret_ps[D:P, (s - 8) * SEG:(s - 7) * SEG], (0, 64)

            # --- seq_fts block: sf[jl, c] for this core's 1024 columns.
            # Borrows ret_ps[:, 0:64] before the main accumulation starts.
            # Evacuation copies on the DVE (idle this early); ACT must stay
            # free for the Exp stream.
            sf_all = singles.tile([P, Q * D], F32)
            for qq in range(Q):
                for kc in range(2):
                    nc.tensor.matmul(
                        ret_ps[:, 0:D],
                        lhsT=xT_sb[:, kc, qq * P:(qq + 1) * P],
                        rhs=w1T_sb[:, kc, :],
                        start=(kc == 0),
                        stop=(kc == 1),
                    )
                nc.vector.tensor_copy(sf_all[:, qq * D:(qq + 1) * D],
                                      ret_ps[:, 0:D])

            sf_scaled = singles.tile([P, Q * D], BF16)
            s16 = singles.tile([P, 16], F32)       # split-row partial sums
            s2 = singles.tile([P, 4], F32)
            sq = singles.tile([P, Q], F32)         # row sums
            rinv = singles.tile([P, Q], F32)       # reciprocals
            # retT_sb[p,:]: p<64 -> retT[p, 0:4096]; p>=64 -> retT[p-64, 4096:]
            ret_sb = singles.tile([P, 8 * SEG], F16)

            # Early rows run Exp at sub-row granularity (streaming right
            # behind the DMA ring, which is the ramp-phase constraint); once
            # the prefetch is ahead of the Exp train, one full-row Exp per
            # row has the least overhead.
            # chunks[q]: (n_chunks, s16 accum slot base).
            chunks = {0: (8, 0), 1: (4, 8), 2: (4, 12)}

            def exp_row(q, b_sb, e_sb):
                if q not in chunks:
                    nc.scalar.activation(
                        out=e_sb, in_=b_sb,
                        func=mybir.ActivationFunctionType.Exp,
                        accum_out=sq[:, q:q + 1],
                    )
                    return
                nch, base = chunks[q]
                w = N // nch
                for c in range(nch):
                    nc.scalar.activation(
                        out=e_sb[:, c * w:(c + 1) * w],
                        in_=b_sb[:, c * w:(c + 1) * w],
                        func=mybir.ActivationFunctionType.Exp,
                        accum_out=s16[:, base + c:base + c + 1],
                    )
                # pairwise-reduce the chunk sums on the DVE
                for c in range(nch // 2):
                    nc.vector.tensor_add(
                        s2[:, c:c + 1],
                        s16[:, base + 2 * c:base + 2 * c + 1],
                        s16[:, base + 2 * c + 1:base + 2 * c + 2],
                    )
                if nch == 8:
                    nc.vector.tensor_add(s2[:, 0:1], s2[:, 0:1], s2[:, 1:2])
                    nc.vector.tensor_add(s2[:, 1:2], s2[:, 2:3], s2[:, 3:4])
                nc.vector.tensor_add(sq[:, q:q + 1], s2[:, 0:1], s2[:, 1:2])

            # --- main loop over j-chunks --------------------------------
            for q in range(Q):
                if q == 0:
                    b_sb = b0_sb
                elif q == 1:
                    b_sb = b1_sb
                else:
                    b_sb = bias_pool.tile([P, N], F16, tag="bias")
                    for h in range(2):
                        nc.sync.dma_start(
                            out=b_sb[:, h * HALF:(h + 1) * HALF],
                            in_=expoP[q * P:(q + 1) * P,
                                      h * HALF:(h + 1) * HALF],
                        )

                e_sb = e_pool.tile([P, N], BF16, tag="e")
                exp_row(q, b_sb, e_sb)

                # normalizer -> fold into the stationary seq_fts weights
                nc.vector.reciprocal(rinv[:, q:q + 1], sq[:, q:q + 1])
                nc.vector.tensor_scalar_mul(
                    sf_scaled[:, q * D:(q + 1) * D],
                    sf_all[:, q * D:(q + 1) * D],
                    rinv[:, q:q + 1],
                )

                # retT[seg] += sf_scaled[q].T @ e[seg]   (sf stationary).
                # On the last q, process segments low/high interleaved,
                # evacuate each PSUM segment right after its final matmul,
                # and kick output DMAs (covering BOTH partition ranges, so
                # both DMA-engine groups stream) halfway and at the end.
                seg_order = (list(range(16)) if q != Q - 1 else
                             [0, 8, 1, 9, 2, 10, 3, 11,
                              4, 12, 5, 13, 6, 14, 7, 15])
                for k, s in enumerate(seg_order):
                    out_ap, tpos = seg_out(s)
                    nc.tensor.matmul(
                        out_ap,
                        lhsT=sf_scaled[:, q * D:(q + 1) * D],
                        rhs=e_sb[:, s * SEG:(s + 1) * SEG],
                        start=(q == 0),
                        stop=(q == Q - 1),
                        tile_position=tpos,
                    )
                    if q == Q - 1:
                        dst = (ret_sb[0:D, s * SEG:(s + 1) * SEG] if s < 8
                               else ret_sb[D:P, (s - 8) * SEG:(s - 7) * SEG])
                        if k % 2 == 0:
                            nc.scalar.copy(out=dst, in_=out_ap)
                        else:
                            nc.vector.tensor_copy(dst, out_ap)
                        if k in (7, 15):
                            c0 = 0 if k == 7 else 4 * SEG
                            nc.sync.dma_start(
                                out=ret[:, c0:c0 + 4 * SEG],
                                in_=ret_sb[0:D, c0:c0 + 4 * SEG],
                            )
                            nc.sync.dma_start(
                                out=ret[:, 8 * SEG + c0:8 * SEG + c0 + 4 * SEG],
                                in_=ret_sb[D:P, c0:c0 + 4 * SEG],
                            )

    if split_multiwaits:
        _split_multiwaits(nc)
    return nc


_NC_CACHE = None


def _get_nc():
    global _NC_CACHE
    if _NC_CACHE is None:
        _NC_CACHE = build_nc()
    return _NC_CACHE


def host_prep(x, bias_mx, W1, a1, a2):
    """Shard + lay out inputs for the 8 cores.

    The transposition pass over each core's bias block folds the whole
    elementwise leaky_relu(f1[j]+f2[i]) logit term, producing the softmax
    exponent matrix the device streams.
    """
    x = np.ascontiguousarray(x, dtype=np.float32)
    W1 = np.ascontiguousarray(W1, dtype=np.float32)
    sf_host = x @ W1.T                   # only used for f1/f2 (logit fold)
    f1 = sf_host @ np.asarray(a1, dtype=np.float32)
    f2 = sf_host @ np.asarray(a2, dtype=np.float32)

    w1T = np.ascontiguousarray(W1.T.astype(NP_BF16))
    in_maps = []
    for d in range(NCORES):
        j0 = d * B
        blk = bias_mx[:, j0:j0 + B]
        z = f1[j0:j0 + B][:, None] + f2[None, :]
        expoP = np.empty((B, N), dtype=np.float32)
        np.copyto(expoP, blk.T)
        expoP += 0.01 * z
        expoP += 0.99 * np.maximum(z, 0.0)
        in_maps.append({
            "expoP": expoP.astype(np.float16),
            "xT": np.ascontiguousarray(x[j0:j0 + B].T.astype(NP_BF16)),
            "w1T": w1T,
        })
    return in_maps


def postprocess(results):
    retT = results[0]["ret"].astype(np.float32)
    for d in range(1, NCORES):
        retT = retT + results[d]["ret"].astype(np.float32)
    r = retT.T
    out = np.where(r > 0.0, r, np.expm1(np.minimum(r, 0.0)))
    return np.ascontiguousarray(out[None], dtype=np.float32)


def kernel(x, bias_mx, W1, a1, a2):
    nc = _get_nc()
    in_maps = host_prep(x, bias_mx, W1, a1, a2)
    res = run_bass_kernel_spmd(nc, in_maps, list(range(NCORES)))
    return postprocess(res.results)


if __name__ == "__main__":
    rng = np.random.default_rng(0)
    x = rng.standard_normal((N, C), dtype=np.float32)
    bias_mx = rng.standard_normal((N, N), dtype=np.float32)
    W1 = rng.standard_normal((D, C), dtype=np.float32) / np.sqrt(C)
    a1 = rng.standard_normal(D).astype(np.float32) / np.sqrt(D)
    a2 = rng.standard_normal(D).astype(np.float32) / np.sqrt(D)
    out = kernel(x=x, bias_mx=bias_mx, W1=W1, a1=a1, a2=a2)
    print("out", out.shape, out.dtype, float(np.abs(out).max()))
